# revision 22
# baseline (speedup 1.0000x reference)
"""Trainium2 Bass kernel for nn_DecoderAttention (4,2048,1024), 16 heads.

Sharding: tensor-parallel over heads. 8 cores = 4 batches x 2 head-groups.
Core i handles batch b=i//2, heads [8g, 8g+8) with g=i%2 (channel slice
[512g, 512g+512)).  Each core computes Q^T/K^T/V projections for its
512 channels, causal attention for its 8 heads (everything kept in the
transposed [channel, token] layout so no on-device transposes are needed),
the residual add, and partial LayerNorm stats; a paired AllReduce
(replica groups [[0,1],[2,3],[4,5],[6,7]]) combines the two half-channel
stats of each batch, then each core normalizes its own slice.  Host
transposes/concatenates the per-core (512, 2048) outputs.

Matmuls run in float32r (tf32-like, full PE rate at N>=512).
"""

import math
import sys

import numpy as np

sys.path.insert(0, "/opt/trn_rl_repo")

import concourse.bacc as bacc  # noqa: E402
import concourse.tile as tile  # noqa: E402
from concourse import mybir  # noqa: E402
from concourse.bass_utils import run_bass_kernel_spmd  # noqa: E402

N_B, N_T, C, NH = 4, 2048, 1024, 16
D = C // NH                      # 64
TP = math.sqrt(2.0 * D)          # temperature
LN_EPS = 1e-5
N_CORES = 8
HPC = 8                          # heads per core
CH = HPC * D                     # channels per core = 512
P = 128
QB = 512                         # q block width
N_QB = N_T // QB                 # 4
N_KB = N_T // P                  # 16 k blocks
KC = C // P                      # 8 contraction chunks
EXP_G = 2                        # k-blocks per exp group (PSUM banks)

f32 = mybir.dt.float32
f32r = mybir.dt.float32r

_PROGRAM_CACHE: dict = {}


def _build_program(mask_mode: str, reps: int = 1, sim_single: bool = False):
    """mask_mode: 'causal' | 'none' | 'general'. reps>1 wraps the compute
    phases in a hardware loop (timing builds only). sim_single builds a
    1-core variant without the collective (for TimelineSim)."""
    nc = bacc.Bacc("TRN2", target_bir_lowering=False, debug=False,
                   num_devices=1 if sim_single else N_CORES)

    xt_d = nc.dram_tensor("xt", [C, N_T], f32r, kind="ExternalInput").ap()
    wq_d = nc.dram_tensor("wq", [C, CH], f32r, kind="ExternalInput").ap()
    wk_d = nc.dram_tensor("wk", [C, CH], f32r, kind="ExternalInput").ap()
    wv_d = nc.dram_tensor("wv", [C, CH], f32r, kind="ExternalInput").ap()
    gam_d = nc.dram_tensor("gam", [CH, 1], f32, kind="ExternalInput").ap()
    bet_d = nc.dram_tensor("bet", [CH, 1], f32, kind="ExternalInput").ap()
    if mask_mode == "general":
        mask_d = nc.dram_tensor("maskt", [N_T, N_T], f32r,
                                kind="ExternalInput").ap()
    yt_d = nc.dram_tensor("yt", [CH, N_T], f32, kind="ExternalOutput").ap()

    with tile.TileContext(nc) as tc:
        _emit(tc, mask_mode, xt_d, wq_d, wk_d, wv_d, gam_d, bet_d,
              mask_d if mask_mode == "general" else None, yt_d, reps=reps,
              sim_single=sim_single)
    nc.compile()
    return nc


def _emit(tc, mask_mode, xt_d, wq_d, wk_d, wv_d, gam_d, bet_d, mask_d, yt_d,
          reps=1, sim_single=False):
    nc = tc.nc
    from contextlib import nullcontext

    def rep_loop():
        return tc.For_i(0, reps, 1) if reps > 1 else nullcontext()
    causal = mask_mode == "causal"
    Exp = mybir.ActivationFunctionType.Exp
    Sqrt = mybir.ActivationFunctionType.Sqrt
    mult = mybir.AluOpType.mult
    add = mybir.AluOpType.add

    from contextlib import ExitStack
    ctx = ExitStack()
    with ctx:
        const = ctx.enter_context(tc.tile_pool(name="const", bufs=1))
        psum = ctx.enter_context(tc.tile_pool(name="psum", bufs=2, space="PSUM"))
        dram = ctx.enter_context(tc.tile_pool(name="dram", bufs=1, space="DRAM"))
        yt_pool = ctx.enter_context(tc.tile_pool(name="yt", bufs=1))

        ones_f32 = const.tile([P, HPC, 1], f32)
        nc.vector.memset(ones_f32, 1.0)
        ones128 = const.tile([P, 1], f32r)
        nc.vector.tensor_copy(out=ones128[:], in_=ones_f32[:, 0, :])

        # yt chunks: core channel rows 128cc..128cc+127, all 2048 tokens
        yt_sb = [yt_pool.tile([P, N_T], f32r, tag=f"yt{cc}", name=f"yt{cc}")
                 for cc in range(4)]

        # ---- attention + projections, phase-scoped pools ----
        with rep_loop(), ExitStack() as actx:
            xt_pool = actx.enter_context(tc.tile_pool(name="xt", bufs=1))
            vp_pool = actx.enter_context(tc.tile_pool(name="vp", bufs=1))
            qk_pool = actx.enter_context(tc.tile_pool(name="qk", bufs=2))
            whp_pool = actx.enter_context(tc.tile_pool(name="whp", bufs=2))
            exp_pool = actx.enter_context(tc.tile_pool(name="expp", bufs=2))
            att_small = actx.enter_context(tc.tile_pool(name="asmall", bufs=1))
            if mask_mode == "general":
                mask_pool = actx.enter_context(tc.tile_pool(name="maskp", bufs=3))

            # ---- V projection for all heads: vp = [128t, kt, 8 heads x 65] ----
            vp = vp_pool.tile([P, N_KB, HPC, D + 1], f32r, tag="vp")
            xt_sb = []
            with tc.tile_pool(name="wv", bufs=1) as wv_pool:
                # interleave wv/x^T loads so the first V matmul can start
                # as soon as (wv[0], xt[0]) land
                wv_sb = []
                for kc in range(KC):
                    t = wv_pool.tile([P, CH], f32r, tag=f"wv{kc}")
                    nc.sync.dma_start(out=t[:], in_=wv_d[kc * P:(kc + 1) * P, :])
                    wv_sb.append(t)
                    tx = xt_pool.tile([P, N_T], f32r, tag=f"xt{kc}",
                                      name=f"xtl{kc}")
                    nc.sync.dma_start(out=tx[:],
                                      in_=xt_d[kc * P:(kc + 1) * P, :])
                    xt_sb.append(tx)
                for kt in range(N_KB):
                    ps = psum.tile([P, QB], f32, tag="proj")
                    for kc in range(KC):
                        nc.tensor.matmul(ps[:],
                                         xt_sb[kc][:, kt * P:(kt + 1) * P],
                                         wv_sb[kc][:], start=(kc == 0),
                                         stop=(kc == KC - 1))
                    nc.vector.tensor_copy(
                        out=vp[:, kt, :, 0:D],
                        in_=ps.rearrange("p (h m) -> p h m", h=HPC))
                    nc.vector.tensor_copy(out=vp[:, kt, :, D:D + 1],
                                          in_=ones_f32[:])

            if causal:
                # additive-mask constants (pre-exp-scale units): trm = 1 where masked
                # (q' < k' + 128r); negI = -600 * I (-53 after the 1/TP
                # exp scale). Folded into the scores
                # matmul as a second accumulate so no post-exp masking is
                # needed.
                trm = actx.enter_context(
                    tc.tile_pool(name="trmp", bufs=1)).tile([P, 4, QB], f32r)
                negI = actx.enter_context(
                    tc.tile_pool(name="negip", bufs=1)).tile([P, P], f32r)
                with tc.tile_pool(name="mstage", bufs=1) as mp:
                    stage = mp.tile([P, 4, QB], f32)
                    nc.vector.memset(stage, 0.0)
                    for r in range(4):
                        # keep-region (q' >= k'+128r) stays 0, else fill 1
                        nc.gpsimd.affine_select(
                            out=stage[:, r, :], in_=stage[:, r, :],
                            pattern=[[1, QB]],
                            compare_op=mybir.AluOpType.is_ge,
                            fill=1.0, base=-(P * r), channel_multiplier=-1)
                    nc.vector.tensor_copy(out=trm[:], in_=stage[:])
                    ident = mp.tile([P, P], f32)
                    nc.vector.memset(ident, 0.0)
                    nc.gpsimd.affine_select(
                        out=ident[:], in_=ident[:], pattern=[[-1, P]],
                        compare_op=mybir.AluOpType.not_equal,
                        fill=1.0, base=0, channel_multiplier=1)
                    nc.scalar.mul(out=ident[:], in_=ident[:], mul=-600.0)
                    nc.vector.tensor_copy(out=negI[:], in_=ident[:])


            inv_tp = 1.0 / TP
            for hp in range(4):  # head pairs
                # per-pair weight slices [128c, 8kc, 128ch]
                wq_hp = whp_pool.tile([P, KC, P], f32r, tag="wq")
                wk_hp = whp_pool.tile([P, KC, P], f32r, tag="wk")
                nc.sync.dma_start(
                    out=wq_hp[:],
                    in_=wq_d.rearrange("(kc p) n -> p kc n", p=P)[
                        :, :, hp * P:(hp + 1) * P])
                nc.sync.dma_start(
                    out=wk_hp[:],
                    in_=wk_d.rearrange("(kc p) n -> p kc n", p=P)[
                        :, :, hp * P:(hp + 1) * P])

                qt_hp = qk_pool.tile([P, N_T], f32r, tag="qt")
                kt_hp = qk_pool.tile([P, N_T], f32r, tag="kt")
                for tb in range(N_QB):
                    psq = psum.tile([P, QB], f32, tag="proj")
                    for kc in range(KC):
                        nc.tensor.matmul(psq[:], wq_hp[:, kc, :],
                                         xt_sb[kc][:, tb * QB:(tb + 1) * QB],
                                         start=(kc == 0), stop=(kc == KC - 1))
                    nc.vector.tensor_copy(out=qt_hp[:, tb * QB:(tb + 1) * QB],
                                          in_=psq[:])
                    psk = psum.tile([P, QB], f32, tag="proj")
                    for kc in range(KC):
                        nc.tensor.matmul(psk[:], wk_hp[:, kc, :],
                                         xt_sb[kc][:, tb * QB:(tb + 1) * QB],
                                         start=(kc == 0), stop=(kc == KC - 1))
                    nc.vector.tensor_copy(out=kt_hp[:, tb * QB:(tb + 1) * QB],
                                          in_=psk[:])

                for hi in range(2):
                    h = 2 * hp + hi
                    kt_h = kt_hp[D * hi:D * hi + D, :]
                    qt_h = qt_hp[D * hi:D * hi + D, :]
                    for qb in range(N_QB):
                        kb_max = 4 * (qb + 1) if causal else N_KB
                        pattn = psum.tile([D + 1, QB], f32, tag="attn")
                        for g0 in range(0, kb_max, EXP_G):
                            gsz = min(EXP_G, kb_max - g0)
                            psc = psum.tile([P, EXP_G, QB], f32, tag="scores")
                            for j in range(gsz):
                                kb = g0 + j
                                r = kb - 4 * qb
                                diag = causal and 0 <= r <= 3
                                nc.tensor.matmul(
                                    psc[:, j, :],
                                    kt_h[:, kb * P:(kb + 1) * P],
                                    qt_h[:, qb * QB:(qb + 1) * QB],
                                    start=True, stop=not diag)
                                if diag:
                                    # scores += -600 where masked
                                    nc.tensor.matmul(
                                        psc[:, j, :], negI[:], trm[:, r, :],
                                        start=False, stop=True)
                            es = exp_pool.tile([P, EXP_G, QB], f32r, tag="es")
                            nc.scalar.activation(out=es[:, 0:gsz, :],
                                                 in_=psc[:, 0:gsz, :],
                                                 func=Exp, scale=inv_tp)
                            for j in range(gsz):
                                kb = g0 + j
                                if mask_mode == "general":
                                    mt = mask_pool.tile([P, QB], f32r, tag="mt")
                                    nc.sync.dma_start(
                                        out=mt[:],
                                        in_=mask_d[kb * P:(kb + 1) * P,
                                                   qb * QB:(qb + 1) * QB])
                                    nc.vector.tensor_mul(es[:, j, :],
                                                         es[:, j, :], mt[:])
                            for j in range(gsz):
                                kb = g0 + j
                                nc.tensor.matmul(
                                    pattn[:], vp[:, kb, h, :], es[:, j, :],
                                    start=(kb == 0), stop=(kb == kb_max - 1))
                        # normalize by sumexp (row D), add residual, store
                        recip = att_small.tile([1, QB], f32, tag="recip")
                        nc.vector.reciprocal(out=recip[:], in_=pattn[D:D + 1, :])
                        bc = att_small.tile([D, QB], f32, tag="bc")
                        nc.gpsimd.partition_broadcast(bc[:], recip[:], D)
                        yslice = yt_sb[hp][D * hi:D * hi + D,
                                           qb * QB:(qb + 1) * QB]
                        nc.vector.tensor_tensor(out=yslice, in0=pattn[0:D, :],
                                                in1=bc[:], op=mult)
                        # residual: x^T rows of this head = xt tile hp,
                        # rows 64*hi.. (host put core's channels first)
                        nc.vector.tensor_add(
                            yslice, yslice,
                            xt_sb[hp][D * hi:D * hi + D,
                                      qb * QB:(qb + 1) * QB])

        # ---- LayerNorm ----
        small = ctx.enter_context(tc.tile_pool(name="small", bufs=1))
        # all row vectors live on partition 0, stacked along a free dim
        stats = small.tile([1, 2, N_T], f32)
        sq_pool = ctx.enter_context(tc.tile_pool(name="sq", bufs=3))
        for tb in range(N_QB):
            s1 = psum.tile([1, QB], f32, tag="proj")
            s2 = psum.tile([1, QB], f32, tag="attn")
            for cc in range(4):
                ysl = yt_sb[cc][:, tb * QB:(tb + 1) * QB]
                nc.tensor.matmul(s1[:], ones128[:], ysl,
                                 start=(cc == 0), stop=(cc == 3))
                sq = sq_pool.tile([P, QB], f32r, tag="sq")
                nc.vector.tensor_mul(sq[:], ysl, ysl)
                nc.tensor.matmul(s2[:], ones128[:], sq[:],
                                 start=(cc == 0), stop=(cc == 3))
            nc.vector.tensor_copy(out=stats[0:1, 0, tb * QB:(tb + 1) * QB],
                                  in_=s1[:])
            nc.vector.tensor_copy(out=stats[0:1, 1, tb * QB:(tb + 1) * QB],
                                  in_=s2[:])

        # pair AllReduce of the stats
        st_in = dram.tile([2, N_T], f32)
        st_out = dram.tile([2, N_T], f32)
        nc.gpsimd.dma_start(out=st_in.unsqueeze(0), in_=stats[:])
        if sim_single:
            nc.gpsimd.dma_start(out=st_out[:], in_=st_in[:])
        else:
            nc.gpsimd.collective_compute(
                "AllReduce", mybir.AluOpType.add,
                replica_groups=[[0, 1], [2, 3], [4, 5], [6, 7]],
                ins=[st_in.opt()], outs=[st_out.opt()])
        statsR = small.tile([1, 2, N_T], f32)
        nc.gpsimd.dma_start(out=statsR[:], in_=st_out.unsqueeze(0))

        mrow = small.tile([1, 2, N_T], f32)   # [0]=mean, [1]=rstd
        mean_r = mrow[0:1, 0, :]
        rstd_r = mrow[0:1, 1, :]
        nc.scalar.mul(out=mean_r, in_=statsR[0:1, 0, :], mul=1.0 / C)
        nc.scalar.mul(out=rstd_r, in_=statsR[0:1, 1, :], mul=1.0 / C)
        msq = small.tile([1, N_T], f32)
        nc.vector.tensor_mul(msq[:], mean_r, mean_r)
        nc.vector.tensor_sub(rstd_r, rstd_r, msq[:])  # var
        eps_t = small.tile([1, 1], f32)
        nc.vector.memset(eps_t, LN_EPS)
        nc.scalar.activation(out=rstd_r, in_=rstd_r, func=Sqrt,
                             bias=eps_t[:])
        nc.vector.reciprocal(out=rstd_r, in_=rstd_r)  # rstd

        # broadcast mean/rstd rows to 128 partitions via K=1 matmuls
        ones1_f32 = small.tile([1, P], f32)
        nc.vector.memset(ones1_f32, 1.0)
        ones1r = small.tile([1, P], f32r)
        nc.vector.tensor_copy(out=ones1r[:], in_=ones1_f32[:])
        mrow_r = small.tile([1, 2, N_T], f32r)
        nc.vector.tensor_copy(out=mrow_r[:], in_=mrow[:])
        meanB = small.tile([P, N_T], f32, tag="meanB")
        rstdB = small.tile([P, N_T], f32, tag="rstdB")
        for tb in range(N_QB):
            pb = psum.tile([P, 2, QB], f32, tag="scores")
            nc.tensor.matmul(pb[:, 0, :], ones1r[:],
                             mrow_r[0:1, 0, tb * QB:(tb + 1) * QB],
                             start=True, stop=True)
            nc.tensor.matmul(pb[:, 1, :], ones1r[:],
                             mrow_r[0:1, 1, tb * QB:(tb + 1) * QB],
                             start=True, stop=True)
            nc.vector.tensor_copy(out=meanB[:, tb * QB:(tb + 1) * QB],
                                  in_=pb[:, 0, :])
            nc.vector.tensor_copy(out=rstdB[:, tb * QB:(tb + 1) * QB],
                                  in_=pb[:, 1, :])

        gam_sb = [const.tile([P, 1], f32, tag=f"gam{cc}", name=f"gam{cc}")
                  for cc in range(4)]
        bet_sb = [const.tile([P, 1], f32, tag=f"bet{cc}", name=f"bet{cc}")
                  for cc in range(4)]
        for cc in range(4):
            nc.sync.dma_start(out=gam_sb[cc][:],
                              in_=gam_d[cc * P:(cc + 1) * P, :])
            nc.sync.dma_start(out=bet_sb[cc][:],
                              in_=bet_d[cc * P:(cc + 1) * P, :])

        out_pool = ctx.enter_context(tc.tile_pool(name="outt", bufs=3))
        for cc in range(4):
            for tb in range(N_QB):
                sl = (slice(None), slice(tb * QB, (tb + 1) * QB))
                ot = out_pool.tile([P, QB], f32, tag="ot")
                nc.vector.tensor_sub(ot[:], yt_sb[cc][sl], meanB[sl])
                nc.vector.tensor_mul(ot[:], ot[:], rstdB[sl])
                nc.vector.tensor_scalar(out=ot[:], in0=ot[:],
                                        scalar1=gam_sb[cc][:],
                                        scalar2=bet_sb[cc][:],
                                        op0=mult, op1=add)
                nc.sync.dma_start(
                    out=yt_d[cc * P:(cc + 1) * P, tb * QB:(tb + 1) * QB],
                    in_=ot[:])


def _detect_mask_mode(mask: np.ndarray) -> str:
    m = np.asarray(mask)
    if not m.any():
        return "none"
    causal = np.triu(np.ones((N_T, N_T), dtype=bool), k=1)
    mb = np.broadcast_to(m, (1, N_B, N_T, N_T))
    if all(np.array_equal(mb[0, b], causal) for b in range(N_B)):
        return "causal"
    return "general"


def _get_program(mask_mode: str):
    if mask_mode not in _PROGRAM_CACHE:
        _PROGRAM_CACHE[mask_mode] = _build_program(mask_mode)
    return _PROGRAM_CACHE[mask_mode]


def _prepare_inputs(x, mask, Wq, Wk, Wv, gamma, beta, mask_mode):
    x = np.ascontiguousarray(np.asarray(x, dtype=np.float32))
    Wq = np.asarray(Wq, dtype=np.float32)
    Wk = np.asarray(Wk, dtype=np.float32)
    Wv = np.asarray(Wv, dtype=np.float32)
    gamma = np.asarray(gamma, dtype=np.float32)
    beta = np.asarray(beta, dtype=np.float32)

    xts = [np.ascontiguousarray(x[b].T) for b in range(N_B)]  # (C, T)
    in_maps = []
    mask_b = None
    if mask_mode == "general":
        m = np.broadcast_to(np.asarray(mask), (1, N_B, N_T, N_T))[0]
        mask_b = [np.ascontiguousarray(
            np.where(m[b], 0.0, 1.0).astype(np.float32).T) for b in range(N_B)]
    for core in range(N_CORES):
        b, g = core // 2, core % 2
        lo, hi = CH * g, CH * g + CH
        # channel permutation: core's own channels first (for the residual)
        perm = np.concatenate([np.arange(lo, hi),
                               np.arange(0, lo), np.arange(hi, C)])
        im = {
            "xt": np.ascontiguousarray(xts[b][perm, :]),
            "wq": np.ascontiguousarray(Wq[lo:hi, perm].T),  # (C, CH) permuted
            "wk": np.ascontiguousarray(Wk[lo:hi, perm].T),
            "wv": np.ascontiguousarray(Wv[lo:hi, perm].T),
            "gam": np.ascontiguousarray(gamma[lo:hi, None]),
            "bet": np.ascontiguousarray(beta[lo:hi, None]),
        }
        if mask_mode == "general":
            im["maskt"] = mask_b[b]
        in_maps.append(im)
    return in_maps


def _run(nc, in_maps):
    return run_bass_kernel_spmd(nc, in_maps, list(range(N_CORES)))


def _assemble(results) -> np.ndarray:
    out = np.empty((N_B, N_T, C), dtype=np.float32)
    for core in range(N_CORES):
        b, g = core // 2, core % 2
        out[b, :, CH * g:CH * g + CH] = results[core]["yt"].T
    return out


def kernel(x, mask, Wq, Wk, Wv, gamma, beta) -> np.ndarray:
    mask_mode = _detect_mask_mode(mask)
    nc = _get_program(mask_mode)
    in_maps = _prepare_inputs(x, mask, Wq, Wk, Wv, gamma, beta, mask_mode)
    res = _run(nc, in_maps)
    return _assemble(res.results)


# revision 31
# speedup vs baseline: 5017.4342x; 5017.4342x over previous
"""Trainium2 Bass kernel for nn_DecoderAttention (4,2048,1024), 16 heads.

Sharding: tensor-parallel over heads. 8 cores = 4 batches x 2 head-groups.
Core i handles batch b=i//2, heads [8g, 8g+8) with g=i%2 (channel slice
[512g, 512g+512)).  Each core computes Q^T/K^T/V projections for its
512 channels, causal attention for its 8 heads (everything kept in the
transposed [channel, token] layout so no on-device transposes are needed),
the residual add, and partial LayerNorm stats; a paired AllReduce
(replica groups [[0,1],[2,3],[4,5],[6,7]]) combines the two half-channel
stats of each batch, then each core normalizes its own slice.  Host
transposes/concatenates the per-core (512, 2048) outputs.

Matmuls run in float32r (tf32-like, full PE rate at N>=512).
"""

import math
import sys

import numpy as np

sys.path.insert(0, "/opt/trn_rl_repo")

import concourse.bacc as bacc  # noqa: E402
import concourse.tile as tile  # noqa: E402
from concourse import mybir  # noqa: E402
from concourse.bass_utils import run_bass_kernel_spmd  # noqa: E402

N_B, N_T, C, NH = 4, 2048, 1024, 16
D = C // NH                      # 64
TP = math.sqrt(2.0 * D)          # temperature
LN_EPS = 1e-5
N_CORES = 8
HPC = 8                          # heads per core
CH = HPC * D                     # channels per core = 512
P = 128
QB = 512                         # q block width
N_QB = N_T // QB                 # 4
N_KB = N_T // P                  # 16 k blocks
KC = C // P                      # 8 contraction chunks
EXP_G = 2                        # k-blocks per exp group (PSUM banks)

f32 = mybir.dt.float32
f32r = mybir.dt.float32r

_PROGRAM_CACHE: dict = {}


DEFAULT_OPTS = dict(mask_impl="affine", qk_bufs=2, whp_bufs=2, exp_bufs=2,
                    pack_pair=False, exp_g=2, scores_bufs=2, ln_bcast="mm",
                    early_stats=False)


def _build_program(mask_mode: str, reps: int = 1, sim_single: bool = False,
                   **opts):
    """mask_mode: 'causal' | 'none' | 'general'. reps>1 wraps the compute
    phases in a hardware loop (timing builds only). sim_single builds a
    1-core variant without the collective (for TimelineSim)."""
    opts = {**DEFAULT_OPTS, **opts}
    nc = bacc.Bacc("TRN2", target_bir_lowering=False, debug=False,
                   num_devices=1 if sim_single else N_CORES)

    xt_d = nc.dram_tensor("xt", [C, N_T], f32r, kind="ExternalInput").ap()
    wq_d = nc.dram_tensor("wq", [C, CH], f32r, kind="ExternalInput").ap()
    wk_d = nc.dram_tensor("wk", [C, CH], f32r, kind="ExternalInput").ap()
    wv_d = nc.dram_tensor("wv", [C, CH], f32r, kind="ExternalInput").ap()
    gam_d = nc.dram_tensor("gam", [CH, 1], f32, kind="ExternalInput").ap()
    bet_d = nc.dram_tensor("bet", [CH, 1], f32, kind="ExternalInput").ap()
    if mask_mode == "general":
        mask_d = nc.dram_tensor("maskt", [N_T, N_T], f32r,
                                kind="ExternalInput").ap()
    yt_d = nc.dram_tensor("yt", [CH, N_T], f32, kind="ExternalOutput").ap()

    with tile.TileContext(nc) as tc:
        _emit(tc, mask_mode, xt_d, wq_d, wk_d, wv_d, gam_d, bet_d,
              mask_d if mask_mode == "general" else None, yt_d, reps=reps,
              sim_single=sim_single, opts=opts)
    nc.compile()
    return nc


def _emit(tc, mask_mode, xt_d, wq_d, wk_d, wv_d, gam_d, bet_d, mask_d, yt_d,
          reps=1, sim_single=False, opts=None):
    opts = {**DEFAULT_OPTS, **(opts or {})}
    mask_mm = opts["mask_impl"] == "mm"
    nc = tc.nc
    from contextlib import nullcontext

    def rep_loop():
        return tc.For_i(0, reps, 1) if reps > 1 else nullcontext()
    causal = mask_mode == "causal"
    Exp = mybir.ActivationFunctionType.Exp
    Sqrt = mybir.ActivationFunctionType.Sqrt
    mult = mybir.AluOpType.mult
    add = mybir.AluOpType.add

    from contextlib import ExitStack
    ctx = ExitStack()
    with ctx:
        const = ctx.enter_context(tc.tile_pool(name="const", bufs=1))
        psum = ctx.enter_context(tc.tile_pool(name="psum", bufs=2, space="PSUM"))
        dram = ctx.enter_context(tc.tile_pool(name="dram", bufs=1, space="DRAM"))
        yt_pool = ctx.enter_context(tc.tile_pool(name="yt", bufs=1))

        ones_f32 = const.tile([P, HPC, 1], f32)
        nc.vector.memset(ones_f32, 1.0)
        ones128 = const.tile([P, 1], f32r)
        nc.vector.tensor_copy(out=ones128[:], in_=ones_f32[:, 0, :])

        # yt chunks: core channel rows 128cc..128cc+127, all 2048 tokens
        yt_sb = [yt_pool.tile([P, N_T], f32r, tag=f"yt{cc}", name=f"yt{cc}")
                 for cc in range(4)]

        stats_early = None
        if opts["early_stats"]:
            statp = ctx.enter_context(tc.tile_pool(name="statp", bufs=1))
            stats_early = statp.tile([1, 2, N_T], f32)
            sqe_pool = ctx.enter_context(tc.tile_pool(name="sqe", bufs=3))

        # ---- attention + projections, phase-scoped pools ----
        with rep_loop(), ExitStack() as actx:
            if stats_early is not None:
                nc.vector.memset(stats_early, 0.0)
            xt_pool = actx.enter_context(tc.tile_pool(name="xt", bufs=1))
            vp_pool = actx.enter_context(tc.tile_pool(name="vp", bufs=1))
            qk_pool = actx.enter_context(tc.tile_pool(name="qk", bufs=opts["qk_bufs"]))
            whp_pool = actx.enter_context(tc.tile_pool(name="whp", bufs=opts["whp_bufs"]))
            exp_pool = actx.enter_context(tc.tile_pool(name="expp", bufs=opts["exp_bufs"]))
            att_small = actx.enter_context(tc.tile_pool(name="asmall", bufs=1))
            if mask_mode == "general":
                mask_pool = actx.enter_context(tc.tile_pool(name="maskp", bufs=3))

            # ---- V projection for all heads: vp = [128t, kt, 8 heads x 65] ----
            vp = vp_pool.tile([P, N_KB, HPC, D + 1], f32r, tag="vp")
            xt_sb = []
            with tc.tile_pool(name="wv", bufs=1) as wv_pool:
                # interleave wv/x^T loads so the first V matmul can start
                # as soon as (wv[0], xt[0]) land
                wv_sb = []
                for kc in range(KC):
                    t = wv_pool.tile([P, CH], f32r, tag=f"wv{kc}")
                    nc.sync.dma_start(out=t[:], in_=wv_d[kc * P:(kc + 1) * P, :])
                    wv_sb.append(t)
                    tx = xt_pool.tile([P, N_T], f32r, tag=f"xt{kc}",
                                      name=f"xtl{kc}")
                    nc.sync.dma_start(out=tx[:],
                                      in_=xt_d[kc * P:(kc + 1) * P, :])
                    xt_sb.append(tx)
                for kt in range(N_KB):
                    ps = psum.tile([P, QB], f32, tag="proj")
                    for kc in range(KC):
                        nc.tensor.matmul(ps[:],
                                         xt_sb[kc][:, kt * P:(kt + 1) * P],
                                         wv_sb[kc][:], start=(kc == 0),
                                         stop=(kc == KC - 1))
                    nc.vector.tensor_copy(
                        out=vp[:, kt, :, 0:D],
                        in_=ps.rearrange("p (h m) -> p h m", h=HPC))
                    nc.vector.tensor_copy(out=vp[:, kt, :, D:D + 1],
                                          in_=ones_f32[:])

            if causal and (mask_mm or opts["mask_impl"] == "mul"):
                # additive-mask constants (pre-exp-scale units): trm = 1 where masked
                # (q' < k' + 128r); negI = -600 * I (-53 after the 1/TP
                # exp scale). Folded into the scores
                # matmul as a second accumulate so no post-exp masking is
                # needed.
                trm = actx.enter_context(
                    tc.tile_pool(name="trmp", bufs=1)).tile([P, 4, QB], f32r)
                negI = None
                if mask_mm:
                    negI = actx.enter_context(
                        tc.tile_pool(name="negip", bufs=1)).tile([P, P], f32r)
                with tc.tile_pool(name="mstage", bufs=1) as mp:
                    stage = mp.tile([P, 4, QB], f32)
                    # mm: masked=1 keep=0 (additive); mul: keep=1 masked=0
                    nc.vector.memset(stage, 0.0 if mask_mm else 1.0)
                    for r in range(4):
                        nc.gpsimd.affine_select(
                            out=stage[:, r, :], in_=stage[:, r, :],
                            pattern=[[1, QB]],
                            compare_op=mybir.AluOpType.is_ge,
                            fill=1.0 if mask_mm else 0.0,
                            base=-(P * r), channel_multiplier=-1)
                    nc.vector.tensor_copy(out=trm[:], in_=stage[:])
                    if mask_mm:
                        ident = mp.tile([P, P], f32)
                        nc.vector.memset(ident, 0.0)
                        nc.gpsimd.affine_select(
                            out=ident[:], in_=ident[:], pattern=[[-1, P]],
                            compare_op=mybir.AluOpType.not_equal,
                            fill=1.0, base=0, channel_multiplier=1)
                        nc.scalar.mul(out=ident[:], in_=ident[:], mul=-600.0)
                        nc.vector.tensor_copy(out=negI[:], in_=ident[:])


            inv_tp = 1.0 / TP
            for hp in range(4):  # head pairs
                # per-pair weight slices [128c, 8kc, 128ch]
                wq_hp = whp_pool.tile([P, KC, P], f32r, tag="wq")
                wk_hp = whp_pool.tile([P, KC, P], f32r, tag="wk")
                nc.sync.dma_start(
                    out=wq_hp[:],
                    in_=wq_d.rearrange("(kc p) n -> p kc n", p=P)[
                        :, :, hp * P:(hp + 1) * P])
                nc.sync.dma_start(
                    out=wk_hp[:],
                    in_=wk_d.rearrange("(kc p) n -> p kc n", p=P)[
                        :, :, hp * P:(hp + 1) * P])

                qt_hp = qk_pool.tile([P, N_T], f32r, tag="qt")
                kt_hp = qk_pool.tile([P, N_T], f32r, tag="kt")
                for tb in range(N_QB):
                    psq = psum.tile([P, QB], f32, tag="proj")
                    for kc in range(KC):
                        nc.tensor.matmul(psq[:], wq_hp[:, kc, :],
                                         xt_sb[kc][:, tb * QB:(tb + 1) * QB],
                                         start=(kc == 0), stop=(kc == KC - 1))
                    nc.vector.tensor_copy(out=qt_hp[:, tb * QB:(tb + 1) * QB],
                                          in_=psq[:])
                    psk = psum.tile([P, QB], f32, tag="proj")
                    for kc in range(KC):
                        nc.tensor.matmul(psk[:], wk_hp[:, kc, :],
                                         xt_sb[kc][:, tb * QB:(tb + 1) * QB],
                                         start=(kc == 0), stop=(kc == KC - 1))
                    nc.vector.tensor_copy(out=kt_hp[:, tb * QB:(tb + 1) * QB],
                                          in_=psk[:])

                if opts["pack_pair"]:
                    # both heads of the pair share the PE array (row groups
                    # 0-63 / 64-127) and one exp instruction per k-block
                    for qb in range(N_QB):
                        kb_max = 4 * (qb + 1) if causal else N_KB
                        pattn2 = [psum.tile([D + 1, QB], f32, tag="attn",
                                            name=f"pattn{hi}")
                                  for hi in range(2)]
                        for kb in range(kb_max):
                            psc = psum.tile([P, 2, QB], f32, tag="scores")
                            for hi in range(2):
                                nc.tensor.matmul(
                                    psc[:, hi, :],
                                    kt_hp[D * hi:D * hi + D,
                                          kb * P:(kb + 1) * P],
                                    qt_hp[D * hi:D * hi + D,
                                          qb * QB:(qb + 1) * QB],
                                    start=True, stop=True)
                            es = exp_pool.tile([P, 2, QB], f32r, tag="es")
                            nc.scalar.activation(out=es[:], in_=psc[:],
                                                 func=Exp, scale=inv_tp)
                            r = kb - 4 * qb
                            if causal and 0 <= r <= 3:
                                for hi in range(2):
                                    nc.gpsimd.affine_select(
                                        out=es[:, hi, :], in_=es[:, hi, :],
                                        pattern=[[1, QB]],
                                        compare_op=mybir.AluOpType.is_ge,
                                        fill=0.0, base=-(P * r),
                                        channel_multiplier=-1)
                            elif mask_mode == "general":
                                mt = mask_pool.tile([P, QB], f32r, tag="mt")
                                nc.sync.dma_start(
                                    out=mt[:],
                                    in_=mask_d[kb * P:(kb + 1) * P,
                                               qb * QB:(qb + 1) * QB])
                                for hi in range(2):
                                    nc.vector.tensor_mul(es[:, hi, :],
                                                         es[:, hi, :], mt[:])
                            for hi in range(2):
                                nc.tensor.matmul(
                                    pattn2[hi][:], vp[:, kb, 2 * hp + hi, :],
                                    es[:, hi, :], start=(kb == 0),
                                    stop=(kb == kb_max - 1))
                        for hi in range(2):
                            pattn = pattn2[hi]
                            recip = att_small.tile([1, QB], f32, tag="recip")
                            nc.vector.reciprocal(out=recip[:],
                                                 in_=pattn[D:D + 1, :])
                            bc = att_small.tile([D, QB], f32, tag="bc")
                            nc.gpsimd.partition_broadcast(bc[:], recip[:], D)
                            yslice = yt_sb[hp][D * hi:D * hi + D,
                                               qb * QB:(qb + 1) * QB]
                            nc.vector.tensor_tensor(out=yslice,
                                                    in0=pattn[0:D, :],
                                                    in1=bc[:], op=mult)
                            nc.vector.tensor_add(
                                yslice, yslice,
                                xt_sb[hp][D * hi:D * hi + D,
                                          qb * QB:(qb + 1) * QB])
                    continue_hi = []
                else:
                    continue_hi = range(2)
                for hi in continue_hi:
                    h = 2 * hp + hi
                    kt_h = kt_hp[D * hi:D * hi + D, :]
                    qt_h = qt_hp[D * hi:D * hi + D, :]
                    for qb in range(N_QB):
                        kb_max = 4 * (qb + 1) if causal else N_KB
                        pattn = psum.tile([D + 1, QB], f32, tag="attn")
                        EG = opts["exp_g"]
                        for g0 in range(0, kb_max, EG):
                            gsz = min(EG, kb_max - g0)
                            psc = psum.tile([P, EG, QB], f32, tag="scores",
                                            bufs=opts["scores_bufs"])
                            for j in range(gsz):
                                kb = g0 + j
                                r = kb - 4 * qb
                                diag = causal and mask_mm and 0 <= r <= 3
                                nc.tensor.matmul(
                                    psc[:, j, :],
                                    kt_h[:, kb * P:(kb + 1) * P],
                                    qt_h[:, qb * QB:(qb + 1) * QB],
                                    start=True, stop=not diag)
                                if diag:
                                    # scores += -600 where masked
                                    nc.tensor.matmul(
                                        psc[:, j, :], negI[:], trm[:, r, :],
                                        start=False, stop=True)
                            es = exp_pool.tile([P, EXP_G, QB], f32r, tag="es")
                            nc.scalar.activation(out=es[:, 0:gsz, :],
                                                 in_=psc[:, 0:gsz, :],
                                                 func=Exp, scale=inv_tp)
                            for j in range(gsz):
                                kb = g0 + j
                                if causal and opts["mask_impl"] == "mul":
                                    r = kb - 4 * qb
                                    if 0 <= r <= 3:
                                        nc.vector.tensor_mul(
                                            es[:, j, :], es[:, j, :],
                                            trm[:, r, :])
                                elif causal and not mask_mm:
                                    r = kb - 4 * qb
                                    if 0 <= r <= 3:
                                        nc.gpsimd.affine_select(
                                            out=es[:, j, :], in_=es[:, j, :],
                                            pattern=[[1, QB]],
                                            compare_op=mybir.AluOpType.is_ge,
                                            fill=0.0, base=-(P * r),
                                            channel_multiplier=-1)
                                elif mask_mode == "general":
                                    mt = mask_pool.tile([P, QB], f32r, tag="mt")
                                    nc.sync.dma_start(
                                        out=mt[:],
                                        in_=mask_d[kb * P:(kb + 1) * P,
                                                   qb * QB:(qb + 1) * QB])
                                    nc.vector.tensor_mul(es[:, j, :],
                                                         es[:, j, :], mt[:])
                            for j in range(gsz):
                                kb = g0 + j
                                nc.tensor.matmul(
                                    pattn[:], vp[:, kb, h, :], es[:, j, :],
                                    start=(kb == 0), stop=(kb == kb_max - 1))
                        # normalize by sumexp (row D), add residual, store
                        recip = att_small.tile([1, QB], f32, tag="recip")
                        nc.vector.reciprocal(out=recip[:], in_=pattn[D:D + 1, :])
                        bc = att_small.tile([D, QB], f32, tag="bc")
                        nc.gpsimd.partition_broadcast(bc[:], recip[:], D)
                        yslice = yt_sb[hp][D * hi:D * hi + D,
                                           qb * QB:(qb + 1) * QB]
                        nc.vector.tensor_tensor(out=yslice, in0=pattn[0:D, :],
                                                in1=bc[:], op=mult)
                        # residual: x^T rows of this head = xt tile hp,
                        # rows 64*hi.. (host put core's channels first)
                        nc.vector.tensor_add(
                            yslice, yslice,
                            xt_sb[hp][D * hi:D * hi + D,
                                      qb * QB:(qb + 1) * QB])

                if stats_early is not None:
                    for tb in range(N_QB):
                        tsl = slice(tb * QB, (tb + 1) * QB)
                        ysl = yt_sb[hp][:, tsl]
                        s1 = psum.tile([1, QB], f32, tag="proj",
                                       name=f"es1_{hp}_{tb}")
                        nc.tensor.matmul(s1[:], ones128[:], ysl,
                                         start=True, stop=True)
                        nc.vector.tensor_add(stats_early[0:1, 0, tsl],
                                             stats_early[0:1, 0, tsl], s1[:])
                        sq = sqe_pool.tile([P, QB], f32r, tag="sqe")
                        nc.vector.tensor_mul(sq[:], ysl, ysl)
                        s2 = psum.tile([1, QB], f32, tag="attn",
                                       name=f"es2_{hp}_{tb}")
                        nc.tensor.matmul(s2[:], ones128[:], sq[:],
                                         start=True, stop=True)
                        nc.vector.tensor_add(stats_early[0:1, 1, tsl],
                                             stats_early[0:1, 1, tsl], s2[:])

        # ---- LayerNorm ----
        small = ctx.enter_context(tc.tile_pool(name="small", bufs=1))
        # all row vectors live on partition 0, stacked along a free dim
        if stats_early is not None:
            stats = stats_early
        else:
            stats = small.tile([1, 2, N_T], f32)
        sq_pool = ctx.enter_context(tc.tile_pool(name="sq", bufs=3))
        if not opts["early_stats"]:
            for tb in range(N_QB):
                s1 = psum.tile([1, QB], f32, tag="proj")
                s2 = psum.tile([1, QB], f32, tag="attn")
                for cc in range(4):
                    ysl = yt_sb[cc][:, tb * QB:(tb + 1) * QB]
                    nc.tensor.matmul(s1[:], ones128[:], ysl,
                                     start=(cc == 0), stop=(cc == 3))
                    sq = sq_pool.tile([P, QB], f32r, tag="sq")
                    nc.vector.tensor_mul(sq[:], ysl, ysl)
                    nc.tensor.matmul(s2[:], ones128[:], sq[:],
                                     start=(cc == 0), stop=(cc == 3))
                nc.vector.tensor_copy(out=stats[0:1, 0, tb * QB:(tb + 1) * QB],
                                      in_=s1[:])
                nc.vector.tensor_copy(out=stats[0:1, 1, tb * QB:(tb + 1) * QB],
                                      in_=s2[:])

        # pair AllReduce of the stats
        st_in = dram.tile([2, N_T], f32)
        st_out = dram.tile([2, N_T], f32)
        nc.gpsimd.dma_start(out=st_in.unsqueeze(0), in_=stats[:])
        if sim_single:
            nc.gpsimd.dma_start(out=st_out[:], in_=st_in[:])
        else:
            nc.gpsimd.collective_compute(
                "AllReduce", mybir.AluOpType.add,
                replica_groups=[[0, 1], [2, 3], [4, 5], [6, 7]],
                ins=[st_in.opt()], outs=[st_out.opt()])
        statsR = small.tile([1, 2, N_T], f32)
        nc.gpsimd.dma_start(out=statsR[:], in_=st_out.unsqueeze(0))

        mrow = small.tile([1, 2, N_T], f32)   # [0]=mean, [1]=rstd
        mean_r = mrow[0:1, 0, :]
        rstd_r = mrow[0:1, 1, :]
        nc.scalar.mul(out=mean_r, in_=statsR[0:1, 0, :], mul=1.0 / C)
        nc.scalar.mul(out=rstd_r, in_=statsR[0:1, 1, :], mul=1.0 / C)
        msq = small.tile([1, N_T], f32)
        nc.vector.tensor_mul(msq[:], mean_r, mean_r)
        nc.vector.tensor_sub(rstd_r, rstd_r, msq[:])  # var
        eps_t = small.tile([1, 1], f32)
        nc.vector.memset(eps_t, LN_EPS)
        nc.scalar.activation(out=rstd_r, in_=rstd_r, func=Sqrt,
                             bias=eps_t[:])
        nc.vector.reciprocal(out=rstd_r, in_=rstd_r)  # rstd

        meanB = small.tile([P, N_T], f32, tag="meanB")
        rstdB = small.tile([P, N_T], f32, tag="rstdB")
        if opts["ln_bcast"] == "mm":
            # broadcast mean/rstd rows to 128 partitions via K=1 matmuls
            ones1_f32 = small.tile([1, P], f32)
            nc.vector.memset(ones1_f32, 1.0)
            ones1r = small.tile([1, P], f32r)
            nc.vector.tensor_copy(out=ones1r[:], in_=ones1_f32[:])
            mrow_r = small.tile([1, 2, N_T], f32r)
            nc.vector.tensor_copy(out=mrow_r[:], in_=mrow[:])
            for tb in range(N_QB):
                pb = psum.tile([P, 2, QB], f32, tag="scores")
                nc.tensor.matmul(pb[:, 0, :], ones1r[:],
                                 mrow_r[0:1, 0, tb * QB:(tb + 1) * QB],
                                 start=True, stop=True)
                nc.tensor.matmul(pb[:, 1, :], ones1r[:],
                                 mrow_r[0:1, 1, tb * QB:(tb + 1) * QB],
                                 start=True, stop=True)
                nc.vector.tensor_copy(out=meanB[:, tb * QB:(tb + 1) * QB],
                                      in_=pb[:, 0, :])
                nc.vector.tensor_copy(out=rstdB[:, tb * QB:(tb + 1) * QB],
                                      in_=pb[:, 1, :])
        else:
            nc.gpsimd.partition_broadcast(meanB[:], mean_r, P)
            nc.gpsimd.partition_broadcast(rstdB[:], rstd_r, P)

        gam_sb = [const.tile([P, 1], f32, tag=f"gam{cc}", name=f"gam{cc}")
                  for cc in range(4)]
        bet_sb = [const.tile([P, 1], f32, tag=f"bet{cc}", name=f"bet{cc}")
                  for cc in range(4)]
        for cc in range(4):
            nc.sync.dma_start(out=gam_sb[cc][:],
                              in_=gam_d[cc * P:(cc + 1) * P, :])
            nc.sync.dma_start(out=bet_sb[cc][:],
                              in_=bet_d[cc * P:(cc + 1) * P, :])

        out_pool = ctx.enter_context(tc.tile_pool(name="outt", bufs=3))
        for cc in range(4):
            for tb in range(N_QB):
                sl = (slice(None), slice(tb * QB, (tb + 1) * QB))
                ot = out_pool.tile([P, QB], f32, tag="ot")
                nc.vector.tensor_sub(ot[:], yt_sb[cc][sl], meanB[sl])
                nc.vector.tensor_mul(ot[:], ot[:], rstdB[sl])
                nc.vector.tensor_scalar(out=ot[:], in0=ot[:],
                                        scalar1=gam_sb[cc][:],
                                        scalar2=bet_sb[cc][:],
                                        op0=mult, op1=add)
                nc.sync.dma_start(
                    out=yt_d[cc * P:(cc + 1) * P, tb * QB:(tb + 1) * QB],
                    in_=ot[:])


def _detect_mask_mode(mask: np.ndarray) -> str:
    m = np.asarray(mask)
    if not m.any():
        return "none"
    causal = np.triu(np.ones((N_T, N_T), dtype=bool), k=1)
    mb = np.broadcast_to(m, (1, N_B, N_T, N_T))
    if all(np.array_equal(mb[0, b], causal) for b in range(N_B)):
        return "causal"
    return "general"


def _get_program(mask_mode: str):
    if mask_mode not in _PROGRAM_CACHE:
        _PROGRAM_CACHE[mask_mode] = _build_program(mask_mode)
    return _PROGRAM_CACHE[mask_mode]


def _prepare_inputs(x, mask, Wq, Wk, Wv, gamma, beta, mask_mode):
    x = np.ascontiguousarray(np.asarray(x, dtype=np.float32))
    Wq = np.asarray(Wq, dtype=np.float32)
    Wk = np.asarray(Wk, dtype=np.float32)
    Wv = np.asarray(Wv, dtype=np.float32)
    gamma = np.asarray(gamma, dtype=np.float32)
    beta = np.asarray(beta, dtype=np.float32)

    xts = [np.ascontiguousarray(x[b].T) for b in range(N_B)]  # (C, T)
    in_maps = []
    mask_b = None
    if mask_mode == "general":
        m = np.broadcast_to(np.asarray(mask), (1, N_B, N_T, N_T))[0]
        mask_b = [np.ascontiguousarray(
            np.where(m[b], 0.0, 1.0).astype(np.float32).T) for b in range(N_B)]
    for core in range(N_CORES):
        b, g = core // 2, core % 2
        lo, hi = CH * g, CH * g + CH
        # channel permutation: core's own channels first (for the residual)
        perm = np.concatenate([np.arange(lo, hi),
                               np.arange(0, lo), np.arange(hi, C)])
        im = {
            "xt": np.ascontiguousarray(xts[b][perm, :]),
            "wq": np.ascontiguousarray(Wq[lo:hi, perm].T),  # (C, CH) permuted
            "wk": np.ascontiguousarray(Wk[lo:hi, perm].T),
            "wv": np.ascontiguousarray(Wv[lo:hi, perm].T),
            "gam": np.ascontiguousarray(gamma[lo:hi, None]),
            "bet": np.ascontiguousarray(beta[lo:hi, None]),
        }
        if mask_mode == "general":
            im["maskt"] = mask_b[b]
        in_maps.append(im)
    return in_maps


def _run(nc, in_maps):
    import time
    last = None
    for attempt in range(3):
        try:
            return run_bass_kernel_spmd(nc, in_maps, list(range(N_CORES)))
        except Exception as e:  # transient NRT_EXEC_UNIT_UNRECOVERABLE
            last = e
            time.sleep(2.0 * (attempt + 1))
    raise last


def _assemble(results) -> np.ndarray:
    out = np.empty((N_B, N_T, C), dtype=np.float32)
    for core in range(N_CORES):
        b, g = core // 2, core % 2
        out[b, :, CH * g:CH * g + CH] = results[core]["yt"].T
    return out


def kernel(x, mask, Wq, Wk, Wv, gamma, beta) -> np.ndarray:
    mask_mode = _detect_mask_mode(mask)
    nc = _get_program(mask_mode)
    in_maps = _prepare_inputs(x, mask, Wq, Wk, Wv, gamma, beta, mask_mode)
    res = _run(nc, in_maps)
    return _assemble(res.results)
